# revision 2
# baseline (speedup 1.0000x reference)
"""CorrelationSampler Trainium2 kernel.

out[b, h, w, c] = bilinear sample of corr[b, :, :, c] at grid position
(h + flow_y, w + flow_x)-ish (align_corners=True, border padding).

Strategy:
  - Host computes integer corner indices and the 4 bilinear weights per
    output position (cheap: B*H*W = 16K positions).
  - Corner indices are re-clamped so ix1 == ix0+1 always (ix0 <= W-2),
    which is mathematically identical to the reference clipping and makes
    the two x-neighbors one contiguous 2*4096-float chunk in memory.
  - 8 cores = batch (4) x position-half (2). Each core gathers row-pairs
    of its batch's [4096, 4096] correlation matrix with indirect DMA and
    blends them on the vector engine with per-partition scalar weights.
"""

import numpy as np

B, H, W = 4, 64, 64
HW = H * W  # 4096 channels; also 4096 source rows per batch
N_CORES = 8
POS_PER_CORE = (B * HW) // N_CORES  # 2048
P = 128  # partitions
N_TILES = POS_PER_CORE // P  # 16


def _host_indices_weights(flow: np.ndarray):
    """float32 replica of the reference's grid math -> corner row indices
    and bilinear corner weights, shape [B, H*W] each."""
    f32 = np.float32
    y_g, x_g = np.meshgrid(
        np.arange(H, dtype=f32), np.arange(W, dtype=f32), indexing="ij"
    )
    x_norm = (f32(2.0) * x_g / f32(W - 1) - f32(1.0)).astype(f32)
    y_norm = (f32(2.0) * y_g / f32(H - 1) - f32(1.0)).astype(f32)

    fx = flow[:, 0].astype(f32)
    fy = flow[:, 1].astype(f32)
    gx = x_norm[None] + fx / f32(W) * f32(2.0)
    gy = y_norm[None] + fy / f32(H) * f32(2.0)

    ix = np.clip((gx + f32(1.0)) * f32(0.5) * f32(W - 1), f32(0.0), f32(W - 1))
    iy = np.clip((gy + f32(1.0)) * f32(0.5) * f32(H - 1), f32(0.0), f32(H - 1))

    # floor is >= 0 after the clip; clamp to W-2/H-2 so the +1 neighbor
    # always exists. At the high border this gives weight 1.0 on the last
    # row/col -- identical result to the reference's clip formulation.
    ix0 = np.minimum(np.floor(ix), f32(W - 2)).astype(np.int32)
    iy0 = np.minimum(np.floor(iy), f32(H - 2)).astype(np.int32)
    wx = (ix - ix0.astype(f32)).astype(f32)
    wy = (iy - iy0.astype(f32)).astype(f32)

    one = f32(1.0)
    w00 = ((one - wy) * (one - wx)).astype(f32)
    w01 = ((one - wy) * wx).astype(f32)
    w10 = (wy * (one - wx)).astype(f32)
    w11 = (wy * wx).astype(f32)

    row0 = iy0 * np.int32(W) + ix0  # gather start row for (iy0, ix0..ix0+1)
    row1 = row0 + np.int32(W)  # (iy0+1, ix0..ix0+1)

    flat = lambda a: a.reshape(B, HW)
    return (
        flat(row0),
        flat(row1),
        flat(w00),
        flat(w01),
        flat(w10),
        flat(w11),
    )


def _build_program():
    import concourse.bacc as bacc
    import concourse.bass as bass
    import concourse.mybir as mybir
    from concourse.tile import TileContext

    f32 = mybir.dt.float32
    i32 = mybir.dt.int32

    nc = bacc.Bacc(
        "TRN2", target_bir_lowering=False, debug=False, num_devices=N_CORES
    )
    corr = nc.dram_tensor("corr", [HW, HW], f32, kind="ExternalInput").ap()
    idx = nc.dram_tensor("idx", [P, 2 * N_TILES], i32, kind="ExternalInput").ap()
    wts = nc.dram_tensor("wts", [P, 4 * N_TILES], f32, kind="ExternalInput").ap()
    out = nc.dram_tensor(
        "out", [POS_PER_CORE, HW], f32, kind="ExternalOutput"
    ).ap()

    mult = mybir.AluOpType.mult
    add = mybir.AluOpType.add

    with TileContext(nc) as tc:
        with (
            tc.tile_pool(name="meta", bufs=1) as meta,
            tc.tile_pool(name="g0", bufs=2) as g0p,
            tc.tile_pool(name="g1", bufs=2) as g1p,
            tc.tile_pool(name="acc", bufs=2) as accp,
        ):
            idx_t = meta.tile([P, 2 * N_TILES], i32)
            wts_t = meta.tile([P, 4 * N_TILES], f32)
            nc.sync.dma_start(out=idx_t[:], in_=idx[:])
            nc.sync.dma_start(out=wts_t[:], in_=wts[:])

            for t in range(N_TILES):
                pair0 = g0p.tile([P, 2 * HW], f32)
                pair1 = g1p.tile([P, 2 * HW], f32)
                nc.gpsimd.indirect_dma_start(
                    out=pair0[:],
                    out_offset=None,
                    in_=corr[:],
                    in_offset=bass.IndirectOffsetOnAxis(
                        ap=idx_t[:, t : t + 1], axis=0
                    ),
                )
                nc.gpsimd.indirect_dma_start(
                    out=pair1[:],
                    out_offset=None,
                    in_=corr[:],
                    in_offset=bass.IndirectOffsetOnAxis(
                        ap=idx_t[:, N_TILES + t : N_TILES + t + 1], axis=0
                    ),
                )
                acc = accp.tile([P, HW], f32)
                # acc = w00*a + w01*b + w10*c + w11*d  (a,b = pair0 halves)
                nc.vector.tensor_scalar_mul(
                    acc[:], pair0[:, 0:HW], wts_t[:, t : t + 1]
                )
                nc.vector.scalar_tensor_tensor(
                    acc[:],
                    pair0[:, HW : 2 * HW],
                    wts_t[:, N_TILES + t : N_TILES + t + 1],
                    acc[:],
                    mult,
                    add,
                )
                nc.vector.scalar_tensor_tensor(
                    acc[:],
                    pair1[:, 0:HW],
                    wts_t[:, 2 * N_TILES + t : 2 * N_TILES + t + 1],
                    acc[:],
                    mult,
                    add,
                )
                nc.vector.scalar_tensor_tensor(
                    acc[:],
                    pair1[:, HW : 2 * HW],
                    wts_t[:, 3 * N_TILES + t : 3 * N_TILES + t + 1],
                    acc[:],
                    mult,
                    add,
                )
                nc.sync.dma_start(out=out[t * P : (t + 1) * P, :], in_=acc[:])
    nc.compile()
    return nc


def _core_meta(row0, row1, w00, w01, w10, w11, b, half):
    """Pack per-core idx [P, 2*N_TILES] and wts [P, 4*N_TILES] tensors.

    Core (b, half) handles flat positions [half*2048, (half+1)*2048) of
    batch b; tile t covers positions half*2048 + t*128 + p."""
    sl = slice(half * POS_PER_CORE, (half + 1) * POS_PER_CORE)
    # [POS_PER_CORE] -> [N_TILES, P] -> [P, N_TILES]
    tp = lambda a: np.ascontiguousarray(a[b, sl].reshape(N_TILES, P).T)
    idx = np.concatenate([tp(row0), tp(row1)], axis=1).astype(np.int32)
    wts = np.concatenate(
        [tp(w00), tp(w01), tp(w10), tp(w11)], axis=1
    ).astype(np.float32)
    return np.ascontiguousarray(idx), np.ascontiguousarray(wts)


_cached = {}


def _get_program():
    if "nc" not in _cached:
        _cached["nc"] = _build_program()
    return _cached["nc"]


def kernel(correlation: np.ndarray, flow: np.ndarray, _trace: bool = False):
    from concourse.bass_utils import run_bass_kernel_spmd

    correlation = np.ascontiguousarray(correlation, dtype=np.float32)
    flow = np.asarray(flow, dtype=np.float32)

    row0, row1, w00, w01, w10, w11 = _host_indices_weights(flow)

    in_maps = []
    for core in range(N_CORES):
        b, half = divmod(core, 2)
        idx, wts = _core_meta(row0, row1, w00, w01, w10, w11, b, half)
        in_maps.append(
            {
                "corr": correlation[b].reshape(HW, HW),
                "idx": idx,
                "wts": wts,
            }
        )

    nc = _get_program()
    res = run_bass_kernel_spmd(
        nc, in_maps, core_ids=list(range(N_CORES)), trace=_trace
    )

    out = np.empty((B, HW, HW), dtype=np.float32)
    for core in range(N_CORES):
        b, half = divmod(core, 2)
        out[b, half * POS_PER_CORE : (half + 1) * POS_PER_CORE, :] = res.results[
            core
        ]["out"]
    if _trace:
        kernel.last_results = res
    return out.reshape(B, H, W, HW)
